# revision 1
# baseline (speedup 1.0000x reference)
"""EntropyDispatchedLinear (int8-weight GEMM with per-column dequant) on 8 TRN2 cores.

out[m, n] = (sum_k x[m, k] * w_int8[k, n]) * w_scale[n],  x fp16 [32, 8192],
w_int8 int8 [8192, 28672], out fp16 [32, 28672].

Strategy (tensor-parallel over out_features N, 3584 columns per core):
- The PE cannot multiply int8 (BIR verifier allows float dtypes only), so the
  weight shard is streamed HBM->SBUF as raw int8 (~29.4 MB at ~320 GB/s under
  8-core HBM contention) and upconverted on-chip to bf16 (exact for int8) by
  the two fast conversion engines in parallel: DVE tensor_copy (cols 0..2176
  of each k-strip, 2x_2P mode ~1.8 elem/ns/partition) and ACT copy (cols
  2176..3584, ~1.15 elem/ns/partition). DMA-cast (gpsimd) was measured slower
  in-kernel (SWDGE descriptor emission + queue interference) and is not used.
- Matmuls: stationary = x^T k-tile [128, 32] fp16 (host-transposed, replicated),
  moving = converted bf16 weight tile [128, 512]. M=32 only fills 32 PE columns,
  so 7 n-tiles are packed into 2 PSUM banks at column offsets 0/32/64/96
  (tile_position col packing) and accumulate over all 64 k-tiles.
- Epilogue: psum * scale (DVE tensor_mul, scale pre-broadcast host-side to the
  packed psum layout), fp16 out, one strided DMA per psum bank.
"""
import numpy as np

M, K, NFULL = 32, 8192, 28672
NCORES = 8
NS = NFULL // NCORES          # 3584 columns per core
KT = K // 128                 # 64 k-tiles
STRIP_KT = 4                  # k-tiles per DMA strip
NSTRIP = KT // STRIP_KT       # 16
DVE_END = 2176                # DVE converts [0, DVE_END), ACT [DVE_END, NS)
NT = NS // 512                # 7 n-tiles

_CACHE = {}


def _build(reps=1):
    import concourse.bacc as bacc
    import concourse.mybir as mybir
    import concourse.tile as tile

    nc = bacc.Bacc("TRN2", target_bir_lowering=False, debug=False, num_devices=NCORES)
    dt = mybir.dt
    xT = nc.dram_tensor("xT", [K, M], dt.float16, kind="ExternalInput").ap()
    w8 = nc.dram_tensor("w8", [K, NS], dt.int8, kind="ExternalInput").ap()
    scaleA = nc.dram_tensor("scaleA", [128, 512], dt.float32, kind="ExternalInput").ap()
    scaleB = nc.dram_tensor("scaleB", [128, 512], dt.float32, kind="ExternalInput").ap()
    out = nc.dram_tensor("out", [M, NS], dt.float16, kind="ExternalOutput").ap()

    w8_t = w8.rearrange("(s t p) n -> s p t n", t=STRIP_KT, p=128)
    xT_t = xT.rearrange("(kt p) m -> p kt m", p=128)

    with tile.TileContext(nc) as tc:
        with (
            tc.tile_pool(name="xp", bufs=1) as xp,
            tc.tile_pool(name="sp", bufs=1) as scp,
            tc.tile_pool(name="wraw", bufs=4) as wrawp,
            tc.tile_pool(name="wbf", bufs=3) as wbfp,
            tc.tile_pool(name="op", bufs=1) as outp,
            tc.tile_pool(name="ps", bufs=1, space="PSUM") as psp,
        ):
            xsb = xp.tile([128, KT, M], dt.float16, tag="x")
            nc.sync.dma_start(xsb[:], xT_t)
            scA = scp.tile([128, 512], dt.float32, tag="scA")
            nc.sync.dma_start(scA[:], scaleA)
            scB = scp.tile([128, 512], dt.float32, tag="scB")
            nc.sync.dma_start(scB[:], scaleB)

            def body():
                pA = psp.tile([128, 512], dt.float32, tag="pA")
                pB = psp.tile([128, 512], dt.float32, tag="pB")
                for s in range(NSTRIP):
                    wraw = wrawp.tile([128, STRIP_KT, NS], dt.int8, tag="wraw")
                    nc.sync.dma_start(wraw[:], w8_t[s])
                    wbf = wbfp.tile([128, STRIP_KT, NS], dt.bfloat16, tag="wbf")
                    for t in range(STRIP_KT):
                        nc.vector.tensor_copy(wbf[:, t, 0:DVE_END], wraw[:, t, 0:DVE_END])
                        nc.scalar.copy(wbf[:, t, DVE_END:NS], wraw[:, t, DVE_END:NS])
                    for t in range(STRIP_KT):
                        kt = s * STRIP_KT + t
                        for nt in range(NT):
                            p, j = (pA, nt) if nt < 4 else (pB, nt - 4)
                            nc.tensor.matmul(
                                p[32 * j:32 * j + 32, :],
                                xsb[:, kt, :],
                                wbf[:, t, 512 * nt:512 * (nt + 1)],
                                start=(kt == 0),
                                stop=(kt == KT - 1),
                                tile_position=(0, 32 * j),
                            )
                oA = outp.tile([128, 512], dt.float16, tag="oA")
                nc.vector.tensor_mul(oA[:], pA[:], scA[:])
                oB = outp.tile([96, 512], dt.float16, tag="oB")
                nc.vector.tensor_mul(oB[:], pB[0:96, :], scB[0:96, :])
                outA_view = out[:, 0:2048].rearrange("m (j f) -> j m f", f=512)
                nc.sync.dma_start(outA_view, oA[:])
                outB_view = out[:, 2048:NS].rearrange("m (j f) -> j m f", f=512)
                nc.sync.dma_start(outB_view, oB[:])

            if reps == 1:
                body()
            else:
                with tc.For_i(0, reps, 1):
                    body()
    nc.compile()
    return nc


def get_nc(reps=1):
    if reps not in _CACHE:
        _CACHE[reps] = _build(reps)
    return _CACHE[reps]


def shard_inputs(x, w_int8, w_scale):
    """Full inputs -> list of 8 per-core input dicts (host-side shard/transpose)."""
    x = np.asarray(x)
    if x.dtype != np.float16:
        x = x.astype(np.float16)
    w_int8 = np.asarray(w_int8)
    if w_int8.dtype != np.int8:
        w_int8 = w_int8.astype(np.int8)
    w_scale = np.asarray(w_scale)
    if w_scale.dtype != np.float32:
        w_scale = w_scale.astype(np.float32)
    x2d = x.reshape(-1, K)
    assert x2d.shape == (M, K), f"unexpected x shape {x.shape}"
    xT = np.ascontiguousarray(x2d.T)
    in_maps = []
    for c in range(NCORES):
        ws = w_scale[c * NS:(c + 1) * NS]
        scA = np.empty((128, 512), np.float32)
        scB = np.zeros((128, 512), np.float32)
        for j in range(4):
            scA[32 * j:32 * j + 32, :] = ws[512 * j:512 * (j + 1)][None, :]
        for j in range(3):
            scB[32 * j:32 * j + 32, :] = ws[2048 + 512 * j:2048 + 512 * (j + 1)][None, :]
        in_maps.append({
            "xT": xT,
            "w8": np.ascontiguousarray(w_int8[:, c * NS:(c + 1) * NS]),
            "scaleA": scA,
            "scaleB": scB,
        })
    return in_maps


def kernel(x, w_int8, w_scale):
    """Full unsharded inputs -> full [32, 28672] fp16 output (8-core TRN2)."""
    from concourse.bass_utils import run_bass_kernel_spmd

    orig_shape = np.asarray(x).shape[:-1] + (NFULL,)
    nc = get_nc(reps=1)
    in_maps = shard_inputs(x, w_int8, w_scale)
    res = run_bass_kernel_spmd(nc, in_maps, core_ids=list(range(NCORES))).results
    out = np.concatenate([res[c]["out"] for c in range(NCORES)], axis=1)
    return out.reshape(orig_shape)
